# revision 1
# baseline (speedup 1.0000x reference)
"""Blocksparse 3x3 conv (16ch -> 16ch, 32x512x512 fp32) on 8 trn2 cores.

Strategy:
- Data-parallel: batch 32 -> 4 images per core.
- Per core, 16 concurrent PE subarray slots: 4 row-groups (one per image)
  x 4 col-groups (4 output rows in flight). Each slot runs K=16 (ic),
  M=16 (oc), N=512 (one output row) matmuls; the 9 conv taps accumulate
  into PSUM via shifted access patterns on the SBUF x tile. fp16 operands
  (host-cast) give 1 cycle/row streaming; fp32 PSUM accumulation.
- x staged in SBUF as [128 partitions, 34 rows, 514] per h-tile: image r
  at partitions 32r..32r+16, rows padded with one halo row each side,
  cols padded with one zero col each side.
- Epilogue: ScalarE copy+bias PSUM -> SBUF staging, batched stores.
"""

import sys

if "/opt/trn_rl_repo" not in sys.path:
    sys.path.insert(0, "/opt/trn_rl_repo")

import numpy as np

N_CORES = 8
IMG_PER_CORE = 4
IC, OC = 16, 16
H, W = 512, 512
HB = 32            # output rows per h-tile
N_HTILES = H // HB
WP = W + 2         # padded row length (zero col each side; f32r needs even N)
ROWS = HB + 2      # rows in x tile (halo)
ROUNDS = HB // 4   # rounds per h-tile (4 rows/round/image)
BAND = HB // 4     # contiguous row band per col group
CHUNK = 4          # rounds per output store chunk

# tap order: full-width center tap first (start=True sets has_written for
# every psum column); edge-kw taps are narrower and accumulate after.
TAPS = [(1, 1)] + [
    (kh, kw) for kh in range(3) for kw in range(3) if (kh, kw) != (1, 1)
]

_BUILD_CACHE = {}


def _build(n_htiles=N_HTILES, it0=0):
    import concourse.bass as bass
    import concourse.bacc as bacc
    import concourse.tile as tile
    from concourse import mybir

    f16 = mybir.dt.float16
    f32 = mybir.dt.float32
    Copy = mybir.ActivationFunctionType.Identity

    nc = bacc.Bacc(trn_type="TRN2")

    x_d = nc.dram_tensor("x", [IMG_PER_CORE, IC, H, W], f16, kind="ExternalInput")
    w_d = nc.dram_tensor("w", [IC, 9, OC], f16, kind="ExternalInput")
    b_d = nc.dram_tensor("b", [128], f32, kind="ExternalInput")
    out_d = nc.dram_tensor(
        "out", [IMG_PER_CORE, OC, H, W], f32, kind="ExternalOutput"
    )

    with tile.TileContext(nc) as tc:
        with (
            tc.tile_pool(name="consts", bufs=1) as consts,
            tc.tile_pool(name="xp", bufs=2) as xp,
            tc.tile_pool(name="sp", bufs=1) as sp,
            tc.tile_pool(name="pp", bufs=8, space="PSUM") as pp,
        ):
            w_sb = consts.tile([128, 9, OC], f16)
            for r in range(4):
                nc.sync.dma_start(out=w_sb[32 * r : 32 * r + IC], in_=w_d[:])
            b_sb = consts.tile([128, 1], f32)
            nc.sync.dma_start(out=b_sb, in_=b_d[:].unsqueeze(1))

            for it in range(it0, it0 + n_htiles):
                h0 = it * HB
                # xa: padded layout, serves kw=0 (offset 0) and kw=2
                # (offset 2) — even element offsets as f32r requires.
                # xb: shifted copy (xb[w] = x[w]), serves kw=1 at offset 0.
                xa = xp.tile([128, ROWS, WP], f16, name="xa")
                xb = xp.tile([128, ROWS, W], f16, name="xb")
                # tile row t holds input row h0-1+t; edge halo rows that
                # fall outside the image are left unwritten and their taps
                # skipped below.
                r_lo = max(0, h0 - 1)
                r_hi = min(H, h0 + HB + 1)
                t_lo = r_lo - (h0 - 1)
                t_hi = t_lo + (r_hi - r_lo)
                if it < it0 + 2:
                    # zero the pad columns once per pool slot; DMA never
                    # writes them, so later h-tiles inherit the zeros
                    nc.vector.memset(xa[:, :, 0:1], 0.0)
                    nc.vector.memset(xa[:, :, WP - 1 : WP], 0.0)
                for r in range(IMG_PER_CORE):
                    nc.sync.dma_start(
                        out=xa[32 * r : 32 * r + IC, t_lo:t_hi, 1 : 1 + W],
                        in_=x_d[r, :, r_lo:r_hi, :],
                    )
                # shifted copy for the center-kw taps (absorbs DMA ticks
                # for its readers)
                nc.gpsimd.tensor_copy(
                    xb[:, t_lo:t_hi, :], xa[:, t_lo:t_hi, 1 : 1 + W]
                )

                # col group c handles rows h0 + BAND*c + j (j = round);
                # stores split into 2 half-chunks of 4 rounds for 3-dim APs
                for half in range(ROUNDS // CHUNK):
                    stages = []
                    for r in range(IMG_PER_CORE):
                        st = sp.tile([128, CHUNK, W], f32, name=f"stage{r}")
                        stages.append(st)
                    for jj in range(CHUNK):
                        j = half * CHUNK + jj
                        psums = []
                        for r in range(IMG_PER_CORE):
                            ps = pp.tile([128, W], f32, name="ps")
                            psums.append(ps)
                        tap_lists = []
                        for c in range(4):
                            row_out = h0 + BAND * c + j
                            tap_lists.append(
                                [
                                    (kh, kw)
                                    for kh, kw in TAPS
                                    if 0 <= row_out - 1 + kh < H
                                ]
                            )
                        max_taps = max(len(t) for t in tap_lists)
                        # wave order: one tap per col group per wave, so all
                        # 16 subarray slots stream concurrently
                        for ti in range(max_taps):
                            for c in range(4):
                                taps = tap_lists[c]
                                if ti >= len(taps):
                                    continue
                                kh, kw = taps[ti]
                                trow = BAND * c + j + kh
                                for r in range(IMG_PER_CORE):
                                    if kw == 1:
                                        rhs = xb[
                                            32 * r : 32 * r + IC, trow, :
                                        ]
                                    else:
                                        rhs = xa[
                                            32 * r : 32 * r + IC,
                                            trow,
                                            kw : kw + W,
                                        ]
                                    nc.tensor.matmul(
                                        out=psums[r][32 * c : 32 * c + OC, :],
                                        lhsT=w_sb[
                                            32 * r : 32 * r + IC,
                                            3 * kh + kw,
                                            :,
                                        ],
                                        rhs=rhs,
                                        start=(ti == 0),
                                        stop=(ti == len(taps) - 1),
                                        tile_position=(32 * r, 32 * c),
                                    )
                        for r in range(IMG_PER_CORE):
                            nc.scalar.activation(
                                out=stages[r][:, jj, :],
                                in_=psums[r][:, :],
                                func=Copy,
                                bias=b_sb[:, 0:1],
                            )
                    # store: img r rows h0 + BAND*c + half*CHUNK + jj
                    pitch = CHUNK * W
                    for r in range(IMG_PER_CORE):
                        st = stages[r]
                        for c in range(4):
                            dst = bass.AP(
                                tensor=out_d,
                                offset=r * (OC * H * W)
                                + (h0 + BAND * c + half * CHUNK) * W,
                                ap=[[H * W, OC], [1, pitch]],
                            )
                            nc.sync.dma_start(
                                out=dst,
                                in_=st[32 * c : 32 * c + OC, :, :],
                            )

    nc.compile()
    return nc


def _get_nc(n_htiles=N_HTILES):
    if n_htiles not in _BUILD_CACHE:
        _BUILD_CACHE[n_htiles] = _build(n_htiles)
    return _BUILD_CACHE[n_htiles]


def _pack_inputs(x, weight, bias, mask):
    wm = (np.asarray(weight) * np.asarray(mask)).astype(np.float32)
    # W_pack[ic, tap, oc] = wm[oc, ic, kh, kw]
    w_pack = np.ascontiguousarray(
        wm.transpose(1, 2, 3, 0).reshape(IC, 9, OC)
    ).astype(np.float16)
    b_pack = np.zeros(128, dtype=np.float32)
    b = np.asarray(bias, dtype=np.float32)
    for c in range(4):
        b_pack[32 * c : 32 * c + OC] = b
    x = np.asarray(x).astype(np.float16)
    in_maps = []
    for i in range(N_CORES):
        in_maps.append(
            {
                "x": np.ascontiguousarray(
                    x[i * IMG_PER_CORE : (i + 1) * IMG_PER_CORE]
                ),
                "w": w_pack,
                "b": b_pack,
            }
        )
    return in_maps


def kernel(x, weight, bias, mask, _trace=False):
    from concourse.bass_utils import run_bass_kernel_spmd

    nc = _get_nc()
    in_maps = _pack_inputs(x, weight, bias, mask)
    res = run_bass_kernel_spmd(
        nc, in_maps, core_ids=list(range(N_CORES)), trace=False
    )
    out = np.concatenate([r["out"] for r in res.results], axis=0)
    return out


def run_timed(x, weight, bias, mask, iters=4):
    """Run the kernel on 8 cores with device-resident inputs, returning
    (full output, best wall-clock ns per iteration). Mirrors
    bass2jax.run_bass_via_pjrt's multi-core path but keeps inputs on
    device so repeat calls time the NEFF execution itself."""
    import time

    import jax
    from jax.experimental.shard_map import shard_map
    from jax.sharding import Mesh, NamedSharding, PartitionSpec

    from concourse import mybir
    from concourse.bass2jax import (
        _bass_exec_p,
        install_neuronx_cc_hook,
        partition_id_tensor,
    )

    install_neuronx_cc_hook()
    nc = _get_nc()
    in_maps = _pack_inputs(x, weight, bias, mask)
    n_cores = N_CORES

    partition_name = (
        nc.partition_id_tensor.name if nc.partition_id_tensor else None
    )
    in_names, out_names, out_avals, zero_outs = [], [], [], []
    for alloc in nc.m.functions[0].allocations:
        if not isinstance(alloc, mybir.MemoryLocationSet):
            continue
        name = alloc.memorylocations[0].name
        if alloc.kind == "ExternalInput":
            if name != partition_name:
                in_names.append(name)
        elif alloc.kind == "ExternalOutput":
            out_names.append(name)
            shape = tuple(alloc.tensor_shape)
            dtype = mybir.dt.np(alloc.dtype)
            out_avals.append(jax.core.ShapedArray(shape, dtype))
            zero_outs.append(np.zeros(shape, dtype))
    n_params = len(in_names)
    n_outs = len(out_avals)
    in_names = in_names + out_names
    if partition_name is not None:
        in_names.append(partition_name)
    donate = tuple(range(n_params, n_params + n_outs))

    def _body(*args):
        operands = list(args)
        if partition_name is not None:
            operands.append(partition_id_tensor())
        outs = _bass_exec_p.bind(
            *operands,
            out_avals=tuple(out_avals),
            in_names=tuple(in_names),
            out_names=tuple(out_names),
            lowering_input_output_aliases=(),
            sim_require_finite=True,
            sim_require_nnan=True,
            nc=nc,
        )
        return tuple(outs)

    devices = jax.devices()[:n_cores]
    mesh = Mesh(np.asarray(devices), ("core",))
    in_specs = (PartitionSpec("core"),) * (n_params + n_outs)
    out_specs = (PartitionSpec("core"),) * len(out_names)
    sharded = jax.jit(
        shard_map(
            _body,
            mesh=mesh,
            in_specs=in_specs,
            out_specs=out_specs,
            check_rep=False,
        ),
        donate_argnums=donate,
        keep_unused=True,
    )
    per_core = [
        [np.asarray(m[name]) for name in in_names[:n_params]] for m in in_maps
    ]
    sh = NamedSharding(mesh, PartitionSpec("core"))
    in_dev = [
        jax.device_put(
            np.concatenate([per_core[c][i] for c in range(n_cores)], axis=0),
            sh,
        )
        for i in range(n_params)
    ]
    concat_zeros = [
        np.zeros((n_cores * z.shape[0], *z.shape[1:]), z.dtype)
        for z in zero_outs
    ]
    best = None
    out_host = None
    for _ in range(iters):
        zeros_dev = [jax.device_put(z, sh) for z in concat_zeros]
        for z in zeros_dev:
            z.block_until_ready()
        for a in in_dev:
            a.block_until_ready()
        t0 = time.perf_counter()
        outs = sharded(*in_dev, *zeros_dev)
        for o in outs:
            o.block_until_ready()
        t1 = time.perf_counter()
        dt_ns = (t1 - t0) * 1e9
        if best is None or dt_ns < best:
            best = dt_ns
            out_host = [np.asarray(o) for o in outs]
    full = out_host[0].reshape(n_cores, IMG_PER_CORE, OC, H, W).reshape(
        n_cores * IMG_PER_CORE, OC, H, W
    )
    return full, best

